# revision 92
# baseline (speedup 1.0000x reference)
"""GQA causal attention (B=4, S=2048, D=2048, H=16, KVH=8, HD=128) on 8 trn2 cores.

Sharding: batch x head-group. Core c = (b, g) with b = c // 2, g = c % 2.
Each core handles one batch and half the heads (8 q-heads, 4 kv-heads),
computing a partial output out_partial = attn_out_g @ wo_g for its batch.
Host sums the two partials per batch (row-sharded wo => partial sums).

Device kernel (per core, identical SPMD program): single pass over the
sequence in 512-wide chunks; K/V/Q projection, rope, attention and output
projection per chunk are pipelined by the Tile scheduler.

Key structure (chosen from cost-model analysis):
  - bf16 everywhere the PE streams; fp32 PSUM accumulation. Halves DMA
    and LDWEIGHTS; rel err ~4e-3 (gate 2e-2).
  - All weights and x are DMA'd via a handful of large descriptors into
    persistent SBUF tiles (host pre-flattens them [128, n*cols] kk-major);
    the HWDGE queue charges per descriptor, so few+large wins.
  - Scores computed transposed (ST[k, q]); causal mask folded into the
    scores matmul: diagonal tiles get a prologue matmul (triT.T @ eye =
    additive -16384 triangle) with start=True, the scores matmul
    accumulates on top. exp underflows masked entries to exactly 0.
  - Fused PV + softmax denominator: P chunks are the *stationary*
    operand, V_aug = [V | ones] the moving operand, accumulating
    [q, HD+1] per (head, q-subtile) in PSUM: attention output and the
    softmax denominator come out of one matmul stream; no separate
    ones-matmul.
  - The [q, d] result is normalized (per-partition reciprocal scale),
    transposed back to [d, q] on the PE (128-col transposes into the
    projection PSUM pool, which is idle during attention), and consumed
    by the output projection in the classic layout.
  - Output projection of chunk sc is emitted interleaved into the
    attention inner loop of chunk sc+1 so the exp (ACT) latency per tile
    is hidden behind PE work.
  - The attention tile stream is software-pipelined two tiles deep
    (scores i+1 and i+2 emitted before the fused matmuls of tile i),
    with score tiles alternating between the psS pool and the psA pool
    (which only holds short-lived transposes during attention). The
    third in-flight score bank absorbs the exp->PE semaphore handoff
    latency that otherwise stalls the PE ~230ns per tile.
"""

import numpy as np

D = 2048
S = 2048
HQ = 8        # q heads per core
HKV = 4       # kv heads per core
HD = 128
KK = D // 128         # 16 contraction subtiles
QC = S // 512         # 4 sequence chunks of 512
NST = S // 128        # 16 sequence tiles of 128
VW = 130              # V storage stride: 128 data + ones col + pad
SCALE = 1.0 / float(np.sqrt(HD))
MASKV = -16384.0      # exact in bf16; exp(SCALE*(s+MASKV)) == 0

_CACHE = {}


def _swap_mask():
    m = []
    for i in range(16):
        m += [2 * i + 1, 2 * i]
    return m


def build_nc():
    """Builds and compiles the per-core Bass program. Returns the Bacc."""
    from contextlib import ExitStack

    import concourse.mybir as mybir
    import concourse.tile as tile
    from concourse import bacc

    f32 = mybir.dt.float32
    bf16 = mybir.dt.bfloat16
    AF = mybir.ActivationFunctionType

    nc = bacc.Bacc(None, target_bir_lowering=False)

    # Host-flattened layouts: [128, ...] with the 128-row subtile index
    # folded into columns, so each tensor loads in O(1) big descriptors.
    xTb = nc.dram_tensor("xTb", [128, QC * KK * 512], bf16, kind="ExternalInput")
    wqb = nc.dram_tensor("wqb", [128, KK * 1024], bf16, kind="ExternalInput")
    wkb = nc.dram_tensor("wkb", [128, KK * 512], bf16, kind="ExternalInput")
    wvb = nc.dram_tensor("wvb", [128, KK * 512], bf16, kind="ExternalInput")
    wob = nc.dram_tensor("wob", [128, 8 * 2048], bf16, kind="ExternalInput")
    cosb = nc.dram_tensor("cosb", [HD, S], f32, kind="ExternalInput")
    sinb = nc.dram_tensor("sinb", [HD, S], f32, kind="ExternalInput")
    tri = nc.dram_tensor("tri", [128, 128], bf16, kind="ExternalInput")
    eye = nc.dram_tensor("eye", [128, 128], f32, kind="ExternalInput")
    out = nc.dram_tensor("out", [S, D], bf16, kind="ExternalOutput")

    SWAP = _swap_mask()

    with tile.TileContext(nc) as tc, ExitStack() as ctx:
        constp = ctx.enter_context(tc.tile_pool(name="constp", bufs=1))
        wp = ctx.enter_context(tc.tile_pool(name="wp", bufs=1))
        kvp = ctx.enter_context(tc.tile_pool(name="kvp", bufs=1))
        vstp = ctx.enter_context(tc.tile_pool(name="vstp", bufs=1))
        tabs = ctx.enter_context(tc.tile_pool(name="tabs", bufs=1))
        xsp = ctx.enter_context(tc.tile_pool(name="xsp", bufs=1))
        tmpp = ctx.enter_context(tc.tile_pool(name="tmpp", bufs=4))
        qtp = ctx.enter_context(tc.tile_pool(name="qtp", bufs=16))
        ptp = ctx.enter_context(tc.tile_pool(name="ptp", bufs=4))
        asp = ctx.enter_context(tc.tile_pool(name="asp", bufs=8))
        rbp = ctx.enter_context(tc.tile_pool(name="rbp", bufs=8))
        onp = ctx.enter_context(tc.tile_pool(name="onp", bufs=16))
        oevp = ctx.enter_context(tc.tile_pool(name="oevp", bufs=2))
        # PSUM pools are bank-granular (every tile is one 2KB bank):
        #   psA: projection accumulators AND [d,q] transposes (time-disjoint)
        #   psS: score tiles; psPV: two [q,129] accumulator regions per bank
        #   psX: out-projection accumulators
        psA = ctx.enter_context(tc.tile_pool(name="psA", bufs=2, space="PSUM"))
        psS = ctx.enter_context(tc.tile_pool(name="psS", bufs=2, space="PSUM"))
        psPV = ctx.enter_context(tc.tile_pool(name="psPV", bufs=2, space="PSUM"))
        psX = ctx.enter_context(tc.tile_pool(name="psX", bufs=2, space="PSUM"))

        # Persistent weights; x double-use tile (refilled per chunk after
        # the projections consumed it). Loads are split into kk-ordered
        # pieces (finest first) so each projection pass can start as soon
        # as its prefix has landed; the DMA queue is strictly serial.
        wkS = wp.tile([128, KK * 512], bf16, name="wkS")
        xS = xsp.tile([128, KK * 512], bf16, name="xS")
        for c0, c1 in [(0, 2), (2, 4), (4, 6), (6, 8), (8, 10), (10, 12), (12, 14), (14, 16)]:
            cs = slice(c0 * 512, c1 * 512)
            nc.sync.dma_start(wkS[:, cs], wkb[:, cs])
            nc.sync.dma_start(xS[:, cs], xTb[:, cs])
        wvS = wp.tile([128, KK * 512], bf16, name="wvS")
        for c0, c1 in [(0, 2), (2, 4), (4, 8), (8, 12), (12, 16)]:
            nc.sync.dma_start(
                wvS[:, c0 * 512:c1 * 512], wvb[:, c0 * 512:c1 * 512]
            )
        wqS = wp.tile([128, KK * 1024], bf16, name="wqS")
        for c0, c1 in [(0, 2), (2, 4), (4, 8), (8, 16)]:
            nc.sync.dma_start(
                wqS[:, c0 * 1024:c1 * 1024], wqb[:, c0 * 1024:c1 * 1024]
            )
        tri_sb = constp.tile([128, 128], bf16, name="tri_sb")
        nc.sync.dma_start(tri_sb[:], tri[:, :])
        eye_f = constp.tile([128, 128], f32, name="eye_f")
        nc.sync.dma_start(eye_f[:], eye[:, :])
        eye_bf = constp.tile([128, 128], bf16, name="eye_bf")
        nc.scalar.copy(eye_bf[:], eye_f[:])
        woS = wp.tile([128, 8 * 2048], bf16, name="woS")
        nc.sync.dma_start(woS[:], wob[:, :])

        # Persistent K^T (rotated) per kv head; V tiles with interleaved
        # ones columns (V_aug layout, stride VW per kv head).
        KT = [
            [kvp.tile([HD, 512], bf16, name=f"kt{h}_{c}") for c in range(QC)]
            for h in range(HKV)
        ]
        V = [vstp.tile([128, HKV * VW], bf16, name=f"v{st}") for st in range(NST)]
        for st in range(NST):
            for kvh in range(HKV):
                nc.vector.memset(V[st][:, kvh * VW + 128:kvh * VW + 129], 1.0)

        def rope(psrc, ct, st, dst):
            """dst = psrc*cos + pairswap(psrc)*sin (dst bf16).

            ACT evicts the PSUM bank in one op (fast release); DVE does the
            rotation from SBUF.
            """
            t1 = tmpp.tile([128, 512], f32, name="rope_t1")
            nc.scalar.copy(t1[:], psrc[:])
            sw = tmpp.tile([128, 512], f32, name="rope_sw")
            nc.vector.stream_shuffle(sw[:], t1[:], SWAP)
            nc.vector.tensor_mul(sw[:], sw[:], st[:])
            nc.vector.tensor_mul(t1[:], t1[:], ct[:])
            nc.vector.tensor_add(dst, t1[:], sw[:])

        def load_tabs(sc):
            ssl = slice(sc * 512, (sc + 1) * 512)
            ct = tabs.tile([HD, 512], f32, name="cos_sl")
            nc.sync.dma_start(ct[:], cosb[:, ssl])
            stt = tabs.tile([HD, 512], f32, name="sin_sl")
            nc.sync.dma_start(stt[:], sinb[:, ssl])
            return ct, stt

        tabs_next = load_tabs(0)
        opj_pending = []        # outproj closures from the previous chunk
        proj1_cls = []
        QTr1 = []
        for sc in range(QC):
            ssl = slice(sc * 512, (sc + 1) * 512)
            ct, stt = tabs_next

            def xa(kk):
                return xS[:, kk * 512:(kk + 1) * 512]

            # K^T projection: 2 passes x 2 kv heads (2 PSUM banks).
            for pp in range(2 if sc != 1 else 0):
                psk = [psA.tile([128, 512], f32, name="psA") for _ in range(2)]
                for kk in range(KK):
                    for i in range(2):
                        h = 2 * pp + i
                        nc.tensor.matmul(
                            psk[i][:],
                            wkS[:, kk * 512 + h * HD:kk * 512 + (h + 1) * HD],
                            xa(kk),
                            start=(kk == 0),
                            stop=(kk == KK - 1),
                        )
                for i in range(2):
                    rope(psk[i], ct, stt, KT[2 * pp + i][sc][:])

            # V projection: 2 passes x 2 seq tiles.
            for pp in range(2 if sc != 1 else 0):
                psv = [psA.tile([128, 512], f32, name="psA") for _ in range(2)]
                for kk in range(KK):
                    for i in range(2):
                        st = 2 * pp + i
                        nc.tensor.matmul(
                            psv[i][:],
                            xa(kk)[:, st * 128:(st + 1) * 128],
                            wvS[:, kk * 512:(kk + 1) * 512],
                            start=(kk == 0),
                            stop=(kk == KK - 1),
                        )
                for i in range(2):
                    st = 2 * pp + i
                    for kvh in range(HKV):
                        # Alternate ACT/DVE so the bank-release chain
                        # halves (gpsimd cannot read PSUM on HW).
                        dst = V[sc * 4 + st][:, kvh * VW:kvh * VW + 128]
                        srcv = psv[i][:, kvh * 128:(kvh + 1) * 128]
                        if kvh % 2 == 0:
                            nc.scalar.copy(dst, srcv)
                        else:
                            nc.vector.tensor_copy(dst, srcv)

            # Q^T projection + rope: 4 passes of 2 heads.
            QTr = QTr1 if sc == 1 else []
            for pp in range(4 if sc != 1 else 0):
                psq = [psA.tile([128, 512], f32, name="psA") for _ in range(2)]
                for kk in range(KK):
                    for i in range(2):
                        h = 2 * pp + i
                        nc.tensor.matmul(
                            psq[i][:],
                            wqS[:, kk * 1024 + h * HD:kk * 1024 + (h + 1) * HD],
                            xa(kk),
                            start=(kk == 0),
                            stop=(kk == KK - 1),
                        )
                for i in range(2):
                    qt = qtp.tile([128, 512], bf16, name="qt")
                    rope(psq[i], ct, stt, qt[:])
                    QTr.append(qt)

            # Refill x for the next chunk (after projections read it) and
            # prefetch the next rope tables.
            if sc + 1 < QC:
                xb = (sc + 1) * KK * 512
                for c0, c1 in [(0, 2), (2, 4), (4, 6), (6, 8), (8, 10), (10, 12), (12, 14), (14, 16)]:
                    nc.sync.dma_start(
                        xS[:, c0 * 512:c1 * 512],
                        xTb[:, xb + c0 * 512:xb + c1 * 512],
                    )
                tabs_next = load_tabs(sc + 1)

            if sc == 0:
                # Chunk 1's projections as pass closures, drained inside
                # attention(0) -- the one attention span with no out-proj
                # filler. They allocate from psX, which holds no out-proj
                # tiles during chunk 0. K passes go last: their KT writes
                # order the remaining chunk-0 score reads after them.
                ct1, stt1 = tabs_next
                s1 = slice(512, 1024)

                def q1_pass(pp):
                    def cl():
                        psq = [psX.tile([128, 512], f32, name="op") for _ in range(2)]
                        for kk in range(KK):
                            for i in range(2):
                                h = 2 * pp + i
                                nc.tensor.matmul(
                                    psq[i][:],
                                    wqS[:, kk * 1024 + h * HD:kk * 1024 + (h + 1) * HD],
                                    xa(kk),
                                    start=(kk == 0),
                                    stop=(kk == KK - 1),
                                )
                        for i in range(2):
                            qt = qtp.tile([128, 512], bf16, name="qt")
                            rope(psq[i], ct1, stt1, qt[:])
                            QTr1.append(qt)
                    return cl

                def v1_pass(pp):
                    def cl():
                        psv = [psX.tile([128, 512], f32, name="op") for _ in range(2)]
                        for kk in range(KK):
                            for i in range(2):
                                st = 2 * pp + i
                                nc.tensor.matmul(
                                    psv[i][:],
                                    xa(kk)[:, st * 128:(st + 1) * 128],
                                    wvS[:, kk * 512:(kk + 1) * 512],
                                    start=(kk == 0),
                                    stop=(kk == KK - 1),
                                )
                        for i in range(2):
                            st = 2 * pp + i
                            for kvh in range(HKV):
                                dst = V[4 + st][:, kvh * VW:kvh * VW + 128]
                                srcv = psv[i][:, kvh * 128:(kvh + 1) * 128]
                                if kvh % 2 == 0:
                                    nc.scalar.copy(dst, srcv)
                                else:
                                    nc.vector.tensor_copy(dst, srcv)
                    return cl

                def k1_pass(pp):
                    def cl():
                        psk = [psX.tile([128, 512], f32, name="op") for _ in range(2)]
                        for kk in range(KK):
                            for i in range(2):
                                h = 2 * pp + i
                                nc.tensor.matmul(
                                    psk[i][:],
                                    wkS[:, kk * 512 + h * HD:kk * 512 + (h + 1) * HD],
                                    xa(kk),
                                    start=(kk == 0),
                                    stop=(kk == KK - 1),
                                )
                        for i in range(2):
                            rope(psk[i], ct1, stt1, KT[2 * pp + i][1][:])
                    return cl

                proj1_cls = [k1_pass(0), q1_pass(0), k1_pass(1)]
                proj1_cls += [q1_pass(p) for p in range(1, 4)]
                proj1_cls += [v1_pass(p) for p in range(2)]

            # ---- Attention, interleaved with previous chunk's out-proj
            # at single-matmul granularity (keeps the PE share per tile
            # uniform so the exp stream on ACT never outpaces the PE).
            nk = 4 * (sc + 1)
            opj_i = 0
            ticks_left = HQ * nk
            ss_par = -1

            proj1_i = 0
            tick_no = 0

            def opj_tick():
                nonlocal opj_i, ticks_left, proj1_i, tick_no
                tick_no += 1
                n = len(opj_pending) - opj_i
                want = -(-n // ticks_left) if ticks_left > 0 else n
                for _ in range(want):
                    opj_pending[opj_i]()
                    opj_i += 1
                ticks_left -= 1
                if (
                    sc == 0
                    and proj1_i < len(proj1_cls)
                    and tick_no >= 6
                    and (tick_no - 6) % 3 == 0
                ):
                    proj1_cls[proj1_i]()
                    proj1_i += 1

            def emit_score(h, kt):
                kvh = h // 2
                j = kt - 4 * sc
                off = 128 * j if j > 0 else 0
                nonlocal ss_par
                ss_par += 1
                if ss_par % 2 == 0:
                    ss = psS.tile([128, 512], f32, name="psS")
                else:
                    ss = psA.tile([128, 512], f32, name="psA")
                if j >= 0:
                    # Scores first (start=True, full region), then the
                    # additive causal triangle accumulates into the 128
                    # diagonal columns. The matmul emits tri[j, i], so
                    # the host supplies the transposed triangle.
                    nc.tensor.matmul(
                        ss[:, off:],
                        KT[kvh][kt >> 2][:, (kt & 3) * 128:((kt & 3) + 1) * 128],
                        QTr[h][:, off:],
                        start=True,
                        stop=False,
                    )
                    nc.tensor.matmul(
                        ss[:, off:off + 128],
                        tri_sb[:],
                        eye_bf[:],
                        start=False,
                        stop=True,
                    )
                else:
                    nc.tensor.matmul(
                        ss[:],
                        KT[kvh][kt >> 2][:, (kt & 3) * 128:((kt & 3) + 1) * 128],
                        QTr[h][:],
                        start=True,
                        stop=True,
                    )
                pt = ptp.tile([128, 512], bf16, name="pt")
                nc.scalar.activation(pt[:, off:], ss[:, off:], AF.Exp, scale=SCALE)
                return pt

            # Flat (h, kt) tile stream, software-pipelined one tile deep
            # across head boundaries so the exp of tile i+1 runs while
            # the PE consumes tile i.
            pending_tp = []
            ON = []
            tiles = [(h, kt) for h in range(HQ) for kt in range(nk)]
            cur = {"atile": None, "on": None}
            pts = {tiles[0]: emit_score(*tiles[0])}
            if len(tiles) > 1:
                pts[tiles[1]] = emit_score(*tiles[1])
            for idx, (h, kt) in enumerate(tiles):
                kvh = h // 2
                j = kt - 4 * sc
                if idx + 2 < len(tiles):
                    pts[tiles[idx + 2]] = emit_score(*tiles[idx + 2])
                pt = pts.pop((h, kt))
                if kt == 0:
                    # Two accumulator banks per head; [q, 129] regions
                    # for (qc0, qc1) at cols 0/256 of bank 0, (qc2, qc3)
                    # in bank 1.
                    cur["atile"] = [
                        psPV.tile([128, 512], f32, name="acc") for _ in range(2)
                    ]
                    cur["on"] = onp.tile([128, 512], bf16, name="on")
                atile, on = cur["atile"], cur["on"]
                for qc in range(max(j, 0), 4):
                    c0 = (qc & 1) * 256
                    nc.tensor.matmul(
                        atile[qc >> 1][:, c0:c0 + 129],
                        pt[:, qc * 128:(qc + 1) * 128],
                        V[kt][:, kvh * VW:kvh * VW + 129],
                        start=(kt == 0 and qc % 2 == 0),
                        stop=(
                            (qc == 1 and kt == 4 * sc + 1)
                            or (qc == 3 and kt == nk - 1)
                        ),
                    )
                if pending_tp:
                    pending_tp.pop(0)()
                opj_tick()
                if j == 1 or j == 3:
                    # Both q-subtiles of acc bank j>>1 are complete:
                    # normalize from PSUM (reciprocal of the fused
                    # denominator column, per-partition scale).
                    for qc in (j - 1, j):
                        c0 = (qc & 1) * 256
                        tl = atile[qc >> 1]
                        rb = rbp.tile([128, 1], f32, name="rb")
                        nc.vector.reciprocal(rb[:], tl[:, c0 + 128:c0 + 129])
                        As = asp.tile([128, 128], f32, name="As")
                        nc.vector.tensor_scalar_mul(
                            As[:], tl[:, c0:c0 + 128], rb[:]
                        )

                        def tp_cl(As=As, on=on, qc=qc):
                            tp = psA.tile([128, 512], f32, name="psA")
                            nc.tensor.transpose(tp[:, :128], As[:], eye_f[:])
                            nc.vector.tensor_copy(
                                on[:, qc * 128:(qc + 1) * 128], tp[:, :128]
                            )

                        pending_tp.append(tp_cl)
                if kt == nk - 1:
                    ON.append(on)
            for cl in pending_tp:
                cl()
            while opj_i < len(opj_pending):
                opj_pending[opj_i]()
                opj_i += 1
            if sc == 0:
                while proj1_i < len(proj1_cls):
                    proj1_cls[proj1_i]()
                    proj1_i += 1

            # ---- Build this chunk's out-proj closures (consumed during
            # chunk sc+1's attention; emitted directly for the last chunk).
            opj_pending = []
            oev_cur = {}
            op_cur = {}

            def make_mm(qs, dc, h2, ONl, sc=sc):
                def mm():
                    if h2 == 0:
                        op_cur[(qs, dc)] = psX.tile([128, 512], f32, name="op")
                    ps = op_cur[(qs, dc)]
                    nc.tensor.matmul(
                        ps[:],
                        ONl[h2][:, qs * 128:(qs + 1) * 128],
                        woS[:, h2 * 2048 + dc * 512:h2 * 2048 + (dc + 1) * 512],
                        start=(h2 == 0),
                        stop=(h2 == HQ - 1),
                    )
                    if h2 == HQ - 1:
                        oev = oevp.tile([128, 512], bf16, name="oev")
                        # Alternate eviction engines in the final
                        # out-proj tail only (ACT is exp-saturated in
                        # the attention-interleaved chunks, but idle at
                        # the very end).
                        if sc == QC - 1 and (qs + dc) % 2 == 1:
                            nc.scalar.copy(oev[:], ps[:])
                        else:
                            nc.vector.tensor_copy(oev[:], ps[:])
                        r0 = sc * 512 + qs * 128
                        nc.sync.dma_start(
                            out[r0:r0 + 128, dc * 512:(dc + 1) * 512], oev[:]
                        )
                return mm

            for qs in range(4):
                for dc in range(4):
                    for h2 in range(HQ):
                        opj_pending.append(make_mm(qs, dc, h2, ON))

            if sc == QC - 1:
                for cl in opj_pending:
                    cl()
                opj_pending = []

    nc.compile()
    return nc


def _get_nc():
    if "nc" not in _CACHE:
        _CACHE["nc"] = build_nc()
    return _CACHE["nc"]


def _flat128(a, rows):
    """[rows*128, C] -> [128, rows*C] with the row-subtile index folded
    into columns (kk-major)."""
    r, c = a.shape
    assert r == rows * 128
    return np.ascontiguousarray(
        a.reshape(rows, 128, c).transpose(1, 0, 2).reshape(128, rows * c)
    )


def _host_prep(x, wq, wk, wv, wo, pos_cos, pos_sin):
    from ml_dtypes import bfloat16

    x = np.asarray(x, dtype=np.float32)
    wq = np.asarray(wq, dtype=np.float32)
    wk = np.asarray(wk, dtype=np.float32)
    wv = np.asarray(wv, dtype=np.float32)
    wo = np.asarray(wo, dtype=np.float32)
    pos_cos = np.asarray(pos_cos, dtype=np.float32)
    pos_sin = np.asarray(pos_sin, dtype=np.float32)

    cosb = np.repeat(pos_cos.T, 2, axis=0).copy()          # [128, S]
    sinb = np.repeat(pos_sin.T, 2, axis=0).copy()          # [128, S]
    sinb[0::2, :] *= -1.0

    # Additive causal mask for a diagonal 128x128 block: the scores
    # matmul adds tri[q, k] to ST[k, q], which must be MASKV iff k > q.
    idx = np.arange(128)
    tri = np.where(idx[:, None] >= idx[None, :], 0.0, MASKV).astype(bfloat16)
    eye = np.eye(128, dtype=np.float32)

    in_maps = []
    for c in range(8):
        b, g = c // 2, c % 2
        # x[b].T is [D, S]; flatten to [128, QC*KK*512] chunk-major then
        # kk-major so one descriptor covers a whole chunk.
        xt = x[b].T.reshape(KK, 128, QC, 512)
        xTb = np.ascontiguousarray(
            xt.transpose(1, 2, 0, 3).reshape(128, QC * KK * 512)
        ).astype(bfloat16)
        in_maps.append({
            "xTb": xTb,
            "wqb": _flat128(wq[:, g * 1024:(g + 1) * 1024], KK).astype(bfloat16),
            "wkb": _flat128(wk[:, g * 512:(g + 1) * 512], KK).astype(bfloat16),
            "wvb": _flat128(wv[:, g * 512:(g + 1) * 512], KK).astype(bfloat16),
            "wob": _flat128(wo[g * 1024:(g + 1) * 1024, :], 8).astype(bfloat16),
            "cosb": cosb,
            "sinb": sinb,
            "tri": tri,
            "eye": eye,
        })
    return in_maps


def kernel(x, wq, wk, wv, wo, pos_cos, pos_sin):
    from concourse.bass_utils import run_bass_kernel_spmd

    nc = _get_nc()
    in_maps = _host_prep(x, wq, wk, wv, wo, pos_cos, pos_sin)
    res = run_bass_kernel_spmd(nc, in_maps, core_ids=list(range(8)))
    outs = [r["out"] for r in res.results]
    full = np.empty((4, S, D), dtype=np.float32)
    for b in range(4):
        full[b] = outs[2 * b].astype(np.float32) + outs[2 * b + 1].astype(np.float32)
    return full
